# revision 34
# baseline (speedup 1.0000x reference)
"""Causal self-attention (dense transformer block) on 8 Trainium2 NeuronCores.

Sharding (Megatron-style tensor parallel over heads):
  - 16 heads, 8 cores -> 2 heads/core. Each core computes the qkv projection
    for its 2 heads (column-sharded W_qkv), causal attention for those heads
    over all 4 batches, and a row-sharded c_proj partial. The host sums the
    8 partial outputs (the row-parallel unshard).
  - Softmax: scores are O(+-6) so exp() without max-subtraction is exact in
    fp32; row sums come free from the PV matmul via a ones-column appended
    to V ([V|1]); causal masking is a 0/1 multiply restricted to the single
    triangular 128-col chunk of each diagonal k-tile.
  - The PE is the bottleneck (and downclocks after idle gaps: 0.65/1.2/2.4
    GHz p-states), so phase 2 is scheduled to keep it continuously busy:
    * k-loop is software-pipelined: scores(kt) issue before PV(kt-1), so
      the exp of kt runs while the PE computes other matmuls.
    * exps split across Scalar ACT (exact, diagonal + some off-diagonal
      tiles) and Vector DVE (Schraudolph bf16-bit exp as int16) so the two
      engines exp concurrently and neither gates the PE.
    * diagonal k-tiles only compute the live q-range [128j, 512): scores,
      exp and PV all shrink; the mask multiply is one [128,2,128] op.
    * y is normalized BEFORE c_proj (sums row broadcast via GpSimd
      partition_broadcast, full-lane DVE reciprocal, two tensor_tensor
      multiplies), so c_proj is a single K=128 f32r matmul per
      (q-chunk, oc-half) -- half the matmuls of the split-head form and no
      PSUM merge arithmetic on the Scalar/Vector engines.
    * c_proj for block i is interleaved into the tail of block i+1's
      k-loop (one z-pair per kt) so PSUM pair slots rotate without stalls.
  - x and the qkv weights are bf16; q/k/v are bf16 downstream. c_proj runs
    in f32r (full PE rate at N=512).
  - Phase 1 (qkv projection) pipelines the V PE-transposes one row-tile
    behind the matmuls so the PE never waits on PSUM evictions.
"""

import sys

sys.path.insert(0, "/opt/trn_rl_repo")

import numpy as np

N_CORES = 8
B, T, D = 4, 2048, 1024
H, DK = 16, 64
HPC = H // N_CORES            # heads per core = 2
CPC = HPC * DK                # channels per core = 128
ROWS = B * T                  # 8192
RT = 512                      # row-tile (free dim) for projections
N_RT = ROWS // RT             # 16
KTILE = 128                   # key tile
QB = 512                      # query block
N_QB = T // QB                # 4 query blocks per batch
N_KT_B = T // KTILE           # 16 key tiles per batch
SCALE = 1.0 / np.sqrt(DK)
# Schraudolph exp for bf16 bit patterns: bf16_bits(exp(x)) ~ A16*x + C16
A16 = 128.0 / np.log(2.0)
C16 = 16252.0  # 127*2^7 with bias correction (halves the sawtooth error)


def round_f32r(x):
    """Round fp32 -> fp32r (round-to-nearest-even at 11 fraction bits)."""
    b = np.ascontiguousarray(x, dtype=np.float32).view(np.uint32)
    r = ((b.astype(np.uint64) + 0x7FF + ((b >> 12) & 1)) & ~np.uint64(0xFFF)).astype(
        np.uint32
    )
    return r.view(np.float32)


def build_program(use_bias=False):
    import concourse.bass as bass  # noqa: F401
    import concourse.mybir as mybir
    import concourse.tile as tile
    from concourse import bacc
    from concourse.masks import make_identity

    f32 = mybir.dt.float32
    f32r = mybir.dt.float32r
    bf16 = mybir.dt.bfloat16
    ACTF = mybir.ActivationFunctionType
    MUL = mybir.AluOpType.mult
    ADD = mybir.AluOpType.add

    nc = bacc.Bacc(None, target_bir_lowering=False)
    with tile.TileContext(nc) as tc:
        with tc.tile_pool(name="dram", bufs=1, space="DRAM") as dram:
            # xT pre-swizzled on host to [p, rt, t, r]; weights to [p, t*m]
            # so every DMA is long contiguous runs per partition
            xT = dram.tile([128, N_RT, D // 128, RT], bf16, kind="ExternalInput", name="xT", uniquify=False)
            wq = dram.tile([128, D], bf16, kind="ExternalInput", name="wq", uniquify=False)
            wk = dram.tile([128, D], bf16, kind="ExternalInput", name="wk", uniquify=False)
            wv = dram.tile([128, D], bf16, kind="ExternalInput", name="wv", uniquify=False)
            wp = dram.tile([CPC, D], f32r, kind="ExternalInput", name="wp", uniquify=False)
            bqkv = dram.tile([CPC, 3], f32, kind="ExternalInput", name="bqkv", uniquify=False)
            bp = dram.tile([1, D], f32, kind="ExternalInput", name="bp", uniquify=False)
            outR = dram.tile([ROWS, D], f32, kind="ExternalOutput", name="outR", uniquify=False)

            # ---------------- constants / weights in SBUF ----------------
            cst = tc.alloc_tile_pool(name="cst", bufs=1)

            # x tiles pool first: issue the rt=0/1 input DMAs before the
            # weight DMAs so the first qkv matmul starts ASAP.
            nkt = D // 128
            xa = tc.alloc_tile_pool(name="xa", bufs=4)
            xt_pre = []
            for rt in range(2):
                rsl = slice(rt * RT, (rt + 1) * RT)
                xt = xa.tile([128, nkt, RT], bf16, name="xt", tag="xt")
                xsrc = xT[:, rt, :, :]
                if rt == 0:
                    # 4 piecewise DMAs: the first 2 k-chunks land fast so the
                    # first matmuls start without waiting for the full tile
                    for c in range(4):
                        nc.sync.dma_start(
                            out=xt[:, 2 * c:2 * c + 2, :],
                            in_=xsrc[:, 2 * c:2 * c + 2, :],
                        )
                else:
                    nc.sync.dma_start(out=xt[:], in_=xsrc)
                xt_pre.append(xt)

            wq_sb = cst.tile([128, D], bf16, name="wq_sb")
            wk_sb = cst.tile([128, D], bf16, name="wk_sb")
            wv_sb = cst.tile([128, D], bf16, name="wv_sb")
            for w_dram, w_sb in ((wq, wq_sb), (wk, wk_sb), (wv, wv_sb)):
                nc.sync.dma_start(out=w_sb[:], in_=w_dram[:])
            wp_sb = cst.tile([CPC, D], f32r, name="wp_sb")
            nc.sync.dma_start(out=wp_sb[:], in_=wp[:])
            bqkv_sb = cst.tile([CPC, 3], f32, name="bqkv_sb")
            nc.sync.dma_start(out=bqkv_sb[:], in_=bqkv[:])
            bp_sb = cst.tile([1, D], f32, name="bp_sb")
            nc.sync.dma_start(out=bp_sb[:], in_=bp[:])
            bp_bc = cst.tile([128, D], f32, name="bp_bc")
            if use_bias:
                nc.gpsimd.partition_broadcast(bp_bc[:], bp_sb[:])

            ident32 = cst.tile([128, 128], f32, name="ident32")
            make_identity(nc, ident32)
            ident = cst.tile([128, 128], f32r, name="ident")
            nc.vector.tensor_copy(ident[:], ident32[:])

            # one [128,128] triangular mask: keep where q >= k (within-chunk)
            tri = cst.tile([128, 128], bf16, name="tri")
            mscratch = cst.tile([128, 128], f32, name="mscratch")
            nc.gpsimd.memset(mscratch[:], 1.0)
            nc.gpsimd.affine_select(
                out=mscratch[:],
                in_=mscratch[:],
                compare_op=mybir.AluOpType.is_ge,
                fill=0.0,
                base=0,
                pattern=[[1, 128]],
                channel_multiplier=-1,
            )
            nc.vector.tensor_copy(tri[:], mscratch[:])

            # ---------------- long-lived activations ----------------
            qt_sb, _free_qt = tc.tile([CPC, ROWS], bf16, name="qt_sb")
            kt_sb, _free_kt = tc.tile([CPC, ROWS], bf16, name="kt_sb")
            # V tiles: per key-tile g: [128 keys, 130]: h0 V|1 at cols 0:65,
            # h1 V|1 at cols 65:130 (ones columns pre-set once)
            v_sb, _free_v = tc.tile([128, (ROWS // KTILE) * 130], bf16, name="v_sb")
            nc.gpsimd.memset(v_sb[:], 1.0)

            # ---------------- pools ----------------
            vts = tc.alloc_tile_pool(name="vts", bufs=3)
            att = tc.alloc_tile_pool(name="att", bufs=10)    # e_t [128,1024]
            ysp = tc.alloc_tile_pool(name="ysp", bufs=3)     # ysb [128,512]
            sbp = tc.alloc_tile_pool(name="sbp", bufs=2)     # srow/rrow rows
            rsp = tc.alloc_tile_pool(name="rsp", bufs=2)     # rr_sb [128,1024]
            osp = tc.alloc_tile_pool(name="osp", bufs=8)     # o_sb [128,1024]
            # phase 1 gets its own 4-slot pair pool (8 banks): q/k/v psum
            # evictions never gate the next row-tile's matmuls
            ps1 = tc.alloc_tile_pool(name="ps1", bufs=4, space="PSUM")

            # ================= phase 1: qkv projections =================
            # V transposes for rt are deferred until after rt+1's matmuls so
            # the PE never waits on the vt eviction chain.
            def emit_transpose(vt_t, rt):
                p_tr = ps1.tile([128, RT], f32r, name="p_tr", tag="pair")
                for c4 in range(RT // 128):
                    nc.tensor.transpose(
                        p_tr[:, c4 * 128:(c4 + 1) * 128],
                        vt_t[:, c4 * 128:(c4 + 1) * 128],
                        ident[:],
                    )
                for c4 in range(RT // 128):
                    g = rt * (RT // 128) + c4
                    base = g * 130
                    nc.vector.tensor_copy(
                        v_sb[:, base:base + 130]
                        .rearrange("p (h c) -> p h c", c=65)[:, :, 0:64],
                        p_tr[:, c4 * 128:(c4 + 1) * 128]
                        .rearrange("p (h c) -> p h c", c=64),
                    )

            pend_tr = None
            for rt in range(N_RT):
                rsl = slice(rt * RT, (rt + 1) * RT)
                if rt < 2:
                    xt = xt_pre[rt]
                else:
                    xt = xa.tile([128, nkt, RT], bf16, name="xt", tag="xt")
                    nc.sync.dma_start(out=xt[:], in_=xT[:, rt, :, :])
                xts = [xt[:, kt, :] for kt in range(nkt)]
                # q and k share one [128,1024] psum pair (separate bank halves)
                p_qk = ps1.tile([CPC, 2 * RT], f32, name="p_qk", tag="pair")
                p_v = ps1.tile([CPC, RT], f32, name="p_v", tag="pair")
                for kt in range(nkt):
                    ksl = slice(kt * 128, (kt + 1) * 128)
                    st = kt == 0
                    sp = kt == nkt - 1
                    nc.tensor.matmul(p_qk[:, 0:RT], wq_sb[:, ksl], xts[kt], start=st, stop=sp)
                    nc.tensor.matmul(p_qk[:, RT:2 * RT], wk_sb[:, ksl], xts[kt], start=st, stop=sp)
                    nc.tensor.matmul(p_v[:], wv_sb[:, ksl], xts[kt], start=st, stop=sp)
                # evict Q^T, K^T
                if use_bias:
                    nc.vector.tensor_scalar_add(qt_sb[:, rsl], p_qk[:, 0:RT], bqkv_sb[:, 0:1])
                    nc.vector.tensor_scalar_add(kt_sb[:, rsl], p_qk[:, RT:2 * RT], bqkv_sb[:, 1:2])
                else:
                    nc.scalar.activation(qt_sb[:, rsl], p_qk[:, 0:RT], ACTF.Copy)
                    nc.vector.tensor_copy(kt_sb[:, rsl], p_qk[:, RT:2 * RT])
                # V^T -> SBUF (with bias); PE-transpose deferred one rt
                vt_t = vts.tile([CPC, RT], f32r, name="vt_t", tag="vt")
                if use_bias:
                    nc.vector.tensor_scalar_add(vt_t[:], p_v[:], bqkv_sb[:, 2:3])
                else:
                    nc.scalar.activation(vt_t[:], p_v[:], ACTF.Copy)
                if pend_tr is not None:
                    emit_transpose(*pend_tr)
                pend_tr = (vt_t, rt)
            emit_transpose(*pend_tr)
            ps1.release()
            # PSUM: pair pool 3 x 2 banks (6) + p_y 2 x 1 bank (2) = 8 banks
            ps_pair = tc.alloc_tile_pool(name="ps_pair", bufs=3, space="PSUM")
            ps_acc = tc.alloc_tile_pool(name="ps_acc", bufs=2, space="PSUM")

            # ================= phase 2: causal attention =================
            # Two-block software pipeline keeps every c_proj dependency at
            # least one full block old by the time the PE reaches it:
            #   norm_queue: (ysb_raw, rrow, row0) -> at kt=0 of the next
            #     block, a PE outer-product broadcasts 1/s into PSUM and two
            #     DVE multiplies normalize ysb in place.
            #   z_queue: (ysb_norm, row0) -> c_proj z-pairs interleaved into
            #     the k-loop one further block later.
            norm_queue = []
            z_queue = []
            recip_pend = []
            ones_f = cst.tile([1, 128], f32, name="ones_f")
            nc.gpsimd.memset(ones_f[:], 1.0)
            ones128 = cst.tile([1, 128], f32r, name="ones128")
            nc.vector.tensor_copy(ones128[:], ones_f[:])

            def emit_norm(ysb, rrow, row0, last=False):
                rr_ps = ps_pair.tile([128, 2 * QB], f32, name="rr_ps", tag="pair")
                nc.tensor.matmul(rr_ps[:, 0:QB], ones128[:], rrow[0:1, 0:QB],
                                 start=True, stop=True)
                nc.tensor.matmul(rr_ps[:, QB:2 * QB], ones128[:], rrow[0:1, QB:2 * QB],
                                 start=True, stop=True)
                if last:
                    # final flush: nothing competes for the PSUM slot or the
                    # Vector queue - multiply straight from PSUM
                    nc.vector.tensor_tensor(
                        out=ysb[0:64, :], in0=ysb[0:64, :],
                        in1=rr_ps[0:64, 0:QB], op=MUL,
                    )
                    nc.vector.tensor_tensor(
                        out=ysb[64:128, :], in0=ysb[64:128, :],
                        in1=rr_ps[64:128, QB:2 * QB], op=MUL,
                    )
                    z_queue.append((ysb, row0))
                    return
                # evict the broadcast to SBUF fast (frees the PSUM slot for
                # the next block's scores) and multiply on GpSimd -- keeps
                # the Vector queue clear for the next block's first exps
                rr_sb = rsp.tile([128, 2 * QB], f32, name="rr_sb", tag="rs")
                nc.scalar.activation(rr_sb[:, 0:QB], rr_ps[:, 0:QB], ACTF.Copy)
                nc.vector.tensor_copy(rr_sb[:, QB:2 * QB], rr_ps[:, QB:2 * QB])
                nc.gpsimd.tensor_tensor(
                    out=ysb[0:64, :], in0=ysb[0:64, :],
                    in1=rr_sb[0:64, 0:QB], op=MUL,
                )
                nc.gpsimd.tensor_tensor(
                    out=ysb[64:128, :], in0=ysb[64:128, :],
                    in1=rr_sb[64:128, QB:2 * QB], op=MUL,
                )
                z_queue.append((ysb, row0))

            def emit_z(y_n, row0, jz):
                z = ps_pair.tile([128, 2 * QB], f32, name="z", tag="pair")
                lhsT = y_n[:, jz * 128:(jz + 1) * 128]
                nc.tensor.matmul(z[:, 0:QB], lhsT, wp_sb[:, 0:QB],
                                 start=True, stop=True)
                nc.tensor.matmul(z[:, QB:2 * QB], lhsT, wp_sb[:, QB:2 * QB],
                                 start=True, stop=True)
                orows = outR[row0 + jz * 128:row0 + (jz + 1) * 128, :]
                o_sb = osp.tile([128, D], f32, name="o_sb", tag="o")
                if jz % 2 == 0:
                    nc.scalar.activation(o_sb[:], z[:], ACTF.Copy)
                else:
                    nc.vector.tensor_copy(o_sb[:], z[:])
                if use_bias:
                    nc.vector.tensor_tensor(
                        out=o_sb[:], in0=o_sb[:], in1=bp_bc[:], op=ADD,
                    )
                nc.sync.dma_start(out=orows, in_=o_sb[:])

            for b in range(B):
                for qb in range(N_QB):
                    qofs = b * T + qb * QB
                    p_y = [
                        ps_acc.tile([65, QB], f32, name=f"p_y{h}", tag="py")
                        for h in range(HPC)
                    ]
                    n_kt = 4 * (qb + 1)
                    zbase = max(n_kt - 4, 2)
                    pv_args = [None] * n_kt

                    def emit_pv(kt):
                        g_, q0_, e_mm_, st_, sp_ = pv_args[kt]
                        for h in range(HPC):
                            vbase = g_ * 130 + h * 65
                            nc.tensor.matmul(
                                p_y[h][:, q0_:QB], v_sb[:, vbase:vbase + 65],
                                e_mm_[:, h, :],
                                start=st_, stop=sp_,
                                skip_group_check=True,
                            )

                    for kt in range(n_kt):
                        g = b * N_KT_B + kt
                        ksl = slice(g * KTILE, (g + 1) * KTILE)
                        diag = kt - 4 * qb  # >= 0 on diagonal tiles
                        q0 = diag * 128 if diag >= 0 else 0
                        st = kt == 0
                        sp = kt == n_kt - 1
                        # both heads' scores -> one [128,1024] pair tile,
                        # restricted to the live q-range on diagonal tiles
                        p_s = ps_pair.tile([128, 2 * QB], f32, name="p_s", tag="pair")
                        nc.tensor.matmul(
                            p_s[:, q0:QB], kt_sb[0:DK, ksl],
                            qt_sb[0:DK, qofs + q0:qofs + QB],
                            start=True, stop=True,
                        )
                        nc.tensor.matmul(
                            p_s[:, QB + q0:2 * QB], kt_sb[DK:CPC, ksl],
                            qt_sb[DK:CPC, qofs + q0:qofs + QB],
                            start=True, stop=True,
                        )
                        # exp over both heads' live range. Engine routing:
                        # the first two kts of every block go to the Vector
                        # engine (Schraudolph bf16-bits exp as int16) so they
                        # are not queued behind the previous block's
                        # eviction+reciprocal chain on Scalar; later
                        # off-diagonal kts alternate Scalar(exact)/Vector,
                        # and later diagonal kts use exact Scalar exp.
                        # Renormalization cancels the approximation error.
                        use_vec = (kt == 0 or diag >= 2
                                   or (diag < 0 and kt % 5 in (1, 3)))
                        if not use_vec:
                            e_t = att.tile([128, 2 * QB], bf16, name="e_t", tag="et")
                            ps3 = p_s[:].rearrange("p (h q) -> p h q", q=QB)[:, :, q0:QB]
                            et3 = e_t[:].rearrange("p (h q) -> p h q", q=QB)[:, :, q0:QB]
                            nc.scalar.activation(et3, ps3, ACTF.Exp, scale=float(SCALE))
                            e_bf = e_t[:]
                        else:
                            e_i = att.tile([128, 2 * QB], mybir.dt.int16, name="e_i", tag="et")
                            if q0 == 0:
                                nc.vector.tensor_scalar(
                                    e_i[:], p_s[:],
                                    float(A16 * SCALE), float(C16),
                                    MUL, ADD,
                                )
                            else:
                                ei3 = e_i[:].rearrange("p (h q) -> p h q", q=QB)[:, :, q0:QB]
                                ps3 = p_s[:].rearrange("p (h q) -> p h q", q=QB)[:, :, q0:QB]
                                nc.vector.tensor_scalar(
                                    ei3, ps3,
                                    float(A16 * SCALE), float(C16),
                                    MUL, ADD,
                                )
                            e_bf = e_i[:].bitcast(bf16)
                        if diag >= 0:
                            # mask the triangular 128-col chunk; for the
                            # first two kts use Vector (same engine as the
                            # Schraudolph exp - no cross-engine hop on the
                            # P0/P1 critical path), else GpSimd
                            etm = e_bf.rearrange("p (h q) -> p h q", q=QB)[
                                :, :, q0:q0 + 128
                            ]
                            meng = nc.vector if kt < 2 else nc.gpsimd
                            meng.tensor_tensor(
                                out=etm,
                                in0=etm,
                                in1=tri[:][:, None, :].broadcast_to([128, HPC, 128]),
                                op=MUL,
                            )
                        e_mm = e_bf.rearrange("p (h q) -> p h q", q=QB)[:, :, q0:QB]
                        pv_args[kt] = (g, q0, e_mm, st, sp)
                        if kt == 1 and recip_pend:
                            srow_, rrow_, rrow_r_ = recip_pend.pop(0)
                            nc.vector.reciprocal_approx_fast(rrow_[:], srow_[:])
                            nc.scalar.activation(rrow_r_[:], rrow_[:], ACTF.Copy)
                        if kt >= 3:
                            emit_pv(kt - 3)
                        jz = kt - zbase
                        if 0 <= jz <= 3 and z_queue:
                            emit_z(z_queue[0][0], z_queue[0][1], jz)
                            if jz == 3:
                                z_queue.pop(0)
                    emit_pv(n_kt - 3)
                    # norm of the previous block here: its reciprocal row is
                    # a full block old, and the two outer-product matmuls
                    # fill the PE while the last exps complete.
                    if norm_queue:
                        emit_norm(*norm_queue.pop(0))
                    njz = n_kt - zbase  # z's already emitted in the k-loop
                    if z_queue and njz < 4:
                        emit_z(z_queue[0][0], z_queue[0][1], njz)
                        njz += 1
                    emit_pv(n_kt - 2)
                    if z_queue and njz < 4:
                        emit_z(z_queue[0][0], z_queue[0][1], njz)
                        njz += 1
                        if njz == 4:
                            z_queue.pop(0)
                    emit_pv(n_kt - 1)

                    # ---- block end: evict y + sums rows, then a 1-pass
                    # Scalar ACT reciprocal on the sums row. Scalar order
                    # [ysb0, srow0, recip] / vector [srow1, ysb1] frees the
                    # p_y banks fast AND finishes the reciprocal before the
                    # deferred outer-product needs it.
                    ysb = ysp.tile([128, QB], f32r, name="ysb", tag="ys")
                    srow = sbp.tile([1, 2 * QB], f32, name="srow", tag="sr")
                    nc.vector.tensor_copy(ysb[64:128, :], p_y[1][0:64, :])
                    nc.scalar.activation(ysb[0:64, :], p_y[0][0:64, :], ACTF.Copy)
                    nc.scalar.activation(srow[0:1, 0:QB], p_y[0][64:65, :], ACTF.Copy)
                    nc.scalar.activation(srow[0:1, QB:2 * QB], p_y[1][64:65, :], ACTF.Copy)
                    rrow = sbp.tile([1, 2 * QB], f32, name="rrow", tag="rr0")
                    rrow_r = sbp.tile([1, 2 * QB], f32r, name="rrow_r", tag="rrr")
                    recip_pend.append((srow, rrow, rrow_r))
                    norm_queue.append((ysb, rrow_r, qofs))

            # tail: flush ready z's first, then the last norm + its z's
            while recip_pend:
                srow_, rrow_, rrow_r_ = recip_pend.pop(0)
                nc.vector.reciprocal_approx_fast(rrow_[:], srow_[:])
                nc.vector.tensor_copy(rrow_r_[:], rrow_[:])
            while z_queue:
                y_, r_ = z_queue.pop(0)
                for jz in range(4):
                    emit_z(y_, r_, jz)
            while norm_queue:
                emit_norm(*norm_queue.pop(0), last=True)
                y_, r_ = z_queue.pop(0)
                for jz in range(4):
                    emit_z(y_, r_, jz)

            for _pool in (ps_acc, ps_pair, osp, rsp, sbp, ysp, att, vts):
                _pool.release()
            _free_v(); _free_kt(); _free_qt()
            xa.release()
            cst.release()

    nc.compile()
    return nc


_CACHED = {}


def _get_program(use_bias=False):
    if use_bias not in _CACHED:
        _CACHED[use_bias] = build_program(use_bias)
    return _CACHED[use_bias]


def make_in_maps(x, W_qkv, b_qkv, W_proj, b_proj):
    x = np.asarray(x, dtype=np.float32)
    W_qkv = np.asarray(W_qkv, dtype=np.float32)
    b_qkv = np.asarray(b_qkv, dtype=np.float32)
    W_proj = np.asarray(W_proj, dtype=np.float32)
    b_proj = np.asarray(b_proj, dtype=np.float32)

    import ml_dtypes

    bf = ml_dtypes.bfloat16
    # [p, rt, t, r]: per-partition rows are 8KB contiguous per rt-tile DMA
    xT = np.ascontiguousarray(
        x.reshape(ROWS, D).T.reshape(D // 128, 128, N_RT, RT)
        .transpose(1, 2, 0, 3)
    ).astype(bf)

    def wswz(w):  # [D, CPC] -> [p, t*m] so the weight DMA is contiguous
        return np.ascontiguousarray(
            w.reshape(D // 128, 128, CPC).transpose(1, 0, 2).reshape(128, D)
        )

    in_maps = []
    for c in range(N_CORES):
        ch = c * CPC  # channel offset of this core's heads
        wq_c = wswz(W_qkv[:, ch:ch + CPC]).astype(bf)
        wk_c = wswz(W_qkv[:, D + ch:D + ch + CPC]).astype(bf)
        wv_c = wswz(W_qkv[:, 2 * D + ch:2 * D + ch + CPC]).astype(bf)
        wp_c = round_f32r(W_proj[ch:ch + CPC, :])
        bqkv_c = np.stack(
            [b_qkv[ch:ch + CPC], b_qkv[D + ch:D + ch + CPC], b_qkv[2 * D + ch:2 * D + ch + CPC]],
            axis=1,
        ).astype(np.float32)
        # b_proj added once (core 0 only); partials are summed on host
        bp_c = (
            b_proj.reshape(1, D)
            if c == 0
            else np.zeros((1, D), np.float32)
        )
        in_maps.append(
            {
                "xT": xT,
                "wq": np.ascontiguousarray(wq_c),
                "wk": np.ascontiguousarray(wk_c),
                "wv": np.ascontiguousarray(wv_c),
                "wp": np.ascontiguousarray(wp_c),
                "bqkv": np.ascontiguousarray(bqkv_c),
                "bp": np.ascontiguousarray(bp_c.astype(np.float32)),
            }
        )
    return in_maps


def run(nc, in_maps, trace=False, trace_kwargs=None):
    from concourse.bass_utils import run_bass_kernel_spmd

    return run_bass_kernel_spmd(
        nc,
        in_maps,
        core_ids=list(range(N_CORES)),
        trace=trace,
        **(trace_kwargs or {}),
    )


def gather_output(results):
    acc = results[0]["outR"].astype(np.float32)
    for r in results[1:]:
        acc = acc + r["outR"]
    return acc.reshape(B, T, D)


def kernel(x, W_qkv, b_qkv, W_proj, b_proj):
    use_bias = bool(np.any(np.asarray(b_qkv)) or np.any(np.asarray(b_proj)))
    nc = _get_program(use_bias)
    in_maps = make_in_maps(x, W_qkv, b_qkv, W_proj, b_proj)
    res = run(nc, in_maps, trace=False)
    return gather_output(res.results)


# revision 36
# speedup vs baseline: 1.0022x; 1.0022x over previous
"""Causal self-attention (dense transformer block) on 8 Trainium2 NeuronCores.

Sharding (Megatron-style tensor parallel over heads):
  - 16 heads, 8 cores -> 2 heads/core. Each core computes the qkv projection
    for its 2 heads (column-sharded W_qkv), causal attention for those heads
    over all 4 batches, and a row-sharded c_proj partial. The host sums the
    8 partial outputs (the row-parallel unshard).
  - Softmax: scores are O(+-6) so exp() without max-subtraction is exact in
    fp32; row sums come free from the PV matmul via a ones-column appended
    to V ([V|1]); causal masking is a 0/1 multiply restricted to the single
    triangular 128-col chunk of each diagonal k-tile.
  - The PE is the bottleneck (and downclocks after idle gaps: 0.65/1.2/2.4
    GHz p-states), so phase 2 is scheduled to keep it continuously busy:
    * k-loop is software-pipelined: scores(kt) issue before PV(kt-1), so
      the exp of kt runs while the PE computes other matmuls.
    * exps split across Scalar ACT (exact, diagonal + some off-diagonal
      tiles) and Vector DVE (Schraudolph bf16-bit exp as int16) so the two
      engines exp concurrently and neither gates the PE.
    * diagonal k-tiles only compute the live q-range [128j, 512): scores,
      exp and PV all shrink; the mask multiply is one [128,2,128] op.
    * y is normalized BEFORE c_proj (sums row broadcast via GpSimd
      partition_broadcast, full-lane DVE reciprocal, two tensor_tensor
      multiplies), so c_proj is a single K=128 f32r matmul per
      (q-chunk, oc-half) -- half the matmuls of the split-head form and no
      PSUM merge arithmetic on the Scalar/Vector engines.
    * c_proj for block i is interleaved into the tail of block i+1's
      k-loop (one z-pair per kt) so PSUM pair slots rotate without stalls.
  - x and the qkv weights are bf16; q/k/v are bf16 downstream. c_proj runs
    in f32r (full PE rate at N=512).
  - Phase 1 (qkv projection) pipelines the V PE-transposes one row-tile
    behind the matmuls so the PE never waits on PSUM evictions.
"""

import sys

sys.path.insert(0, "/opt/trn_rl_repo")

import numpy as np

N_CORES = 8
B, T, D = 4, 2048, 1024
H, DK = 16, 64
HPC = H // N_CORES            # heads per core = 2
CPC = HPC * DK                # channels per core = 128
ROWS = B * T                  # 8192
RT = 512                      # row-tile (free dim) for projections
N_RT = ROWS // RT             # 16
KTILE = 128                   # key tile
QB = 512                      # query block
N_QB = T // QB                # 4 query blocks per batch
N_KT_B = T // KTILE           # 16 key tiles per batch
SCALE = 1.0 / np.sqrt(DK)
# Schraudolph exp for bf16 bit patterns: bf16_bits(exp(x)) ~ A16*x + C16
A16 = 128.0 / np.log(2.0)
C16 = 16252.0  # 127*2^7 with bias correction (halves the sawtooth error)


def round_f32r(x):
    """Round fp32 -> fp32r (round-to-nearest-even at 11 fraction bits)."""
    b = np.ascontiguousarray(x, dtype=np.float32).view(np.uint32)
    r = ((b.astype(np.uint64) + 0x7FF + ((b >> 12) & 1)) & ~np.uint64(0xFFF)).astype(
        np.uint32
    )
    return r.view(np.float32)


def build_program(use_bias=False):
    import concourse.bass as bass  # noqa: F401
    import concourse.mybir as mybir
    import concourse.tile as tile
    from concourse import bacc
    from concourse.masks import make_identity

    f32 = mybir.dt.float32
    f32r = mybir.dt.float32r
    bf16 = mybir.dt.bfloat16
    ACTF = mybir.ActivationFunctionType
    MUL = mybir.AluOpType.mult
    ADD = mybir.AluOpType.add

    nc = bacc.Bacc(None, target_bir_lowering=False)
    with tile.TileContext(nc) as tc:
        with tc.tile_pool(name="dram", bufs=1, space="DRAM") as dram:
            # xT pre-swizzled on host to [p, rt, t, r]; weights to [p, t*m]
            # so every DMA is long contiguous runs per partition
            xT = dram.tile([128, N_RT, D // 128, RT], bf16, kind="ExternalInput", name="xT", uniquify=False)
            wq = dram.tile([128, D], bf16, kind="ExternalInput", name="wq", uniquify=False)
            wk = dram.tile([128, D], bf16, kind="ExternalInput", name="wk", uniquify=False)
            wv = dram.tile([128, D], bf16, kind="ExternalInput", name="wv", uniquify=False)
            wp = dram.tile([CPC, D], f32r, kind="ExternalInput", name="wp", uniquify=False)
            bqkv = dram.tile([CPC, 3], f32, kind="ExternalInput", name="bqkv", uniquify=False)
            bp = dram.tile([1, D], f32, kind="ExternalInput", name="bp", uniquify=False)
            outR = dram.tile([ROWS, D], f32, kind="ExternalOutput", name="outR", uniquify=False)

            # ---------------- constants / weights in SBUF ----------------
            cst = tc.alloc_tile_pool(name="cst", bufs=1)

            # x tiles pool first: issue the rt=0/1 input DMAs before the
            # weight DMAs so the first qkv matmul starts ASAP.
            nkt = D // 128
            xa = tc.alloc_tile_pool(name="xa", bufs=3)
            xt_pre = []
            for rt in range(2):
                rsl = slice(rt * RT, (rt + 1) * RT)
                xt = xa.tile([128, nkt, RT], bf16, name="xt", tag="xt")
                xsrc = xT[:, rt, :, :]
                if rt == 0:
                    # 4 piecewise DMAs: the first 2 k-chunks land fast so the
                    # first matmuls start without waiting for the full tile
                    for c in range(4):
                        nc.sync.dma_start(
                            out=xt[:, 2 * c:2 * c + 2, :],
                            in_=xsrc[:, 2 * c:2 * c + 2, :],
                        )
                else:
                    nc.sync.dma_start(out=xt[:], in_=xsrc)
                xt_pre.append(xt)

            wq_sb = cst.tile([128, D], bf16, name="wq_sb")
            wk_sb = cst.tile([128, D], bf16, name="wk_sb")
            wv_sb = cst.tile([128, D], bf16, name="wv_sb")
            # halves: the first 4 k-chunks of each weight land first so the
            # leading qkv matmuls never wait on the full 256KB load
            for w_dram, w_sb in ((wq, wq_sb), (wk, wk_sb), (wv, wv_sb)):
                nc.sync.dma_start(out=w_sb[:, 0:D // 2], in_=w_dram[:, 0:D // 2])
            for w_dram, w_sb in ((wq, wq_sb), (wk, wk_sb), (wv, wv_sb)):
                nc.sync.dma_start(out=w_sb[:, D // 2:D], in_=w_dram[:, D // 2:D])
            wp_sb = cst.tile([CPC, D], f32r, name="wp_sb")
            nc.sync.dma_start(out=wp_sb[:], in_=wp[:])
            bqkv_sb = cst.tile([CPC, 3], f32, name="bqkv_sb")
            nc.sync.dma_start(out=bqkv_sb[:], in_=bqkv[:])
            bp_sb = cst.tile([1, D], f32, name="bp_sb")
            nc.sync.dma_start(out=bp_sb[:], in_=bp[:])
            bp_bc = cst.tile([128, D], f32, name="bp_bc")
            if use_bias:
                nc.gpsimd.partition_broadcast(bp_bc[:], bp_sb[:])

            ident32 = cst.tile([128, 128], f32, name="ident32")
            make_identity(nc, ident32)
            ident = cst.tile([128, 128], f32r, name="ident")
            nc.vector.tensor_copy(ident[:], ident32[:])

            # one [128,128] triangular mask: keep where q >= k (within-chunk)
            tri = cst.tile([128, 128], bf16, name="tri")
            mscratch = cst.tile([128, 128], f32, name="mscratch")
            nc.gpsimd.memset(mscratch[:], 1.0)
            nc.gpsimd.affine_select(
                out=mscratch[:],
                in_=mscratch[:],
                compare_op=mybir.AluOpType.is_ge,
                fill=0.0,
                base=0,
                pattern=[[1, 128]],
                channel_multiplier=-1,
            )
            nc.vector.tensor_copy(tri[:], mscratch[:])

            # ---------------- long-lived activations ----------------
            qt_sb, _free_qt = tc.tile([CPC, ROWS], bf16, name="qt_sb")
            kt_sb, _free_kt = tc.tile([CPC, ROWS], bf16, name="kt_sb")
            # V tiles: per key-tile g: [128 keys, 130]: h0 V|1 at cols 0:65,
            # h1 V|1 at cols 65:130 (ones columns pre-set once)
            v_sb, _free_v = tc.tile([128, (ROWS // KTILE) * 130], bf16, name="v_sb")
            nc.gpsimd.memset(v_sb[:], 1.0)

            # ---------------- pools ----------------
            vts = tc.alloc_tile_pool(name="vts", bufs=3)
            att = tc.alloc_tile_pool(name="att", bufs=8)     # e_t [128,1024]
            ysp = tc.alloc_tile_pool(name="ysp", bufs=3)     # ysb [128,512]
            sbp = tc.alloc_tile_pool(name="sbp", bufs=2)     # srow/rrow rows
            rsp = tc.alloc_tile_pool(name="rsp", bufs=2)     # rr_sb [128,1024]
            osp = tc.alloc_tile_pool(name="osp", bufs=6)     # o_sb [128,1024]
            # phase 1 gets its own 4-slot pair pool (8 banks): q/k/v psum
            # evictions never gate the next row-tile's matmuls
            ps1 = tc.alloc_tile_pool(name="ps1", bufs=4, space="PSUM")

            # ================= phase 1: qkv projections =================
            # V transposes for rt are deferred until after rt+1's matmuls so
            # the PE never waits on the vt eviction chain.
            def emit_transpose(vt_t, rt):
                p_tr = ps1.tile([128, RT], f32r, name="p_tr", tag="pair")
                for c4 in range(RT // 128):
                    nc.tensor.transpose(
                        p_tr[:, c4 * 128:(c4 + 1) * 128],
                        vt_t[:, c4 * 128:(c4 + 1) * 128],
                        ident[:],
                    )
                for c4 in range(RT // 128):
                    g = rt * (RT // 128) + c4
                    base = g * 130
                    nc.vector.tensor_copy(
                        v_sb[:, base:base + 130]
                        .rearrange("p (h c) -> p h c", c=65)[:, :, 0:64],
                        p_tr[:, c4 * 128:(c4 + 1) * 128]
                        .rearrange("p (h c) -> p h c", c=64),
                    )

            pend_tr = None
            for rt in range(N_RT):
                rsl = slice(rt * RT, (rt + 1) * RT)
                if rt < 2:
                    xt = xt_pre[rt]
                else:
                    xt = xa.tile([128, nkt, RT], bf16, name="xt", tag="xt")
                    nc.sync.dma_start(out=xt[:], in_=xT[:, rt, :, :])
                xts = [xt[:, kt, :] for kt in range(nkt)]
                # q and k share one [128,1024] psum pair (separate bank halves)
                p_qk = ps1.tile([CPC, 2 * RT], f32, name="p_qk", tag="pair")
                p_v = ps1.tile([CPC, RT], f32, name="p_v", tag="pair")
                for kt in range(nkt):
                    ksl = slice(kt * 128, (kt + 1) * 128)
                    st = kt == 0
                    sp = kt == nkt - 1
                    nc.tensor.matmul(p_qk[:, 0:RT], wq_sb[:, ksl], xts[kt], start=st, stop=sp)
                    nc.tensor.matmul(p_qk[:, RT:2 * RT], wk_sb[:, ksl], xts[kt], start=st, stop=sp)
                    nc.tensor.matmul(p_v[:], wv_sb[:, ksl], xts[kt], start=st, stop=sp)
                # evict Q^T, K^T
                if use_bias:
                    nc.vector.tensor_scalar_add(qt_sb[:, rsl], p_qk[:, 0:RT], bqkv_sb[:, 0:1])
                    nc.vector.tensor_scalar_add(kt_sb[:, rsl], p_qk[:, RT:2 * RT], bqkv_sb[:, 1:2])
                else:
                    nc.scalar.activation(qt_sb[:, rsl], p_qk[:, 0:RT], ACTF.Copy)
                    nc.vector.tensor_copy(kt_sb[:, rsl], p_qk[:, RT:2 * RT])
                # V^T -> SBUF (with bias); PE-transpose deferred one rt
                vt_t = vts.tile([CPC, RT], f32r, name="vt_t", tag="vt")
                if use_bias:
                    nc.vector.tensor_scalar_add(vt_t[:], p_v[:], bqkv_sb[:, 2:3])
                else:
                    nc.scalar.activation(vt_t[:], p_v[:], ACTF.Copy)
                if pend_tr is not None:
                    emit_transpose(*pend_tr)
                pend_tr = (vt_t, rt)
            emit_transpose(*pend_tr)
            ps1.release()
            # PSUM: pair pool 3 x 2 banks (6) + p_y 2 x 1 bank (2) = 8 banks
            ps_pair = tc.alloc_tile_pool(name="ps_pair", bufs=3, space="PSUM")
            ps_acc = tc.alloc_tile_pool(name="ps_acc", bufs=2, space="PSUM")

            # ================= phase 2: causal attention =================
            # Two-block software pipeline keeps every c_proj dependency at
            # least one full block old by the time the PE reaches it:
            #   norm_queue: (ysb_raw, rrow, row0) -> at kt=0 of the next
            #     block, a PE outer-product broadcasts 1/s into PSUM and two
            #     DVE multiplies normalize ysb in place.
            #   z_queue: (ysb_norm, row0) -> c_proj z-pairs interleaved into
            #     the k-loop one further block later.
            norm_queue = []
            z_queue = []
            recip_pend = []
            ones_f = cst.tile([1, 128], f32, name="ones_f")
            nc.gpsimd.memset(ones_f[:], 1.0)
            ones128 = cst.tile([1, 128], f32r, name="ones128")
            nc.vector.tensor_copy(ones128[:], ones_f[:])

            def emit_norm(ysb, rrow, row0):
                rr_ps = ps_pair.tile([128, 2 * QB], f32, name="rr_ps", tag="pair")
                nc.tensor.matmul(rr_ps[:, 0:QB], ones128[:], rrow[0:1, 0:QB],
                                 start=True, stop=True)
                nc.tensor.matmul(rr_ps[:, QB:2 * QB], ones128[:], rrow[0:1, QB:2 * QB],
                                 start=True, stop=True)
                # evict the broadcast to SBUF fast (frees the PSUM slot for
                # the next block's scores) and multiply on GpSimd -- keeps
                # the Vector queue clear for the next block's first exps
                rr_sb = rsp.tile([128, 2 * QB], f32, name="rr_sb", tag="rs")
                nc.scalar.activation(rr_sb[:, 0:QB], rr_ps[:, 0:QB], ACTF.Copy)
                nc.vector.tensor_copy(rr_sb[:, QB:2 * QB], rr_ps[:, QB:2 * QB])
                nc.gpsimd.tensor_tensor(
                    out=ysb[0:64, :], in0=ysb[0:64, :],
                    in1=rr_sb[0:64, 0:QB], op=MUL,
                )
                nc.gpsimd.tensor_tensor(
                    out=ysb[64:128, :], in0=ysb[64:128, :],
                    in1=rr_sb[64:128, QB:2 * QB], op=MUL,
                )
                z_queue.append((ysb, row0))

            def emit_z(y_n, row0, jz):
                z = ps_pair.tile([128, 2 * QB], f32, name="z", tag="pair")
                lhsT = y_n[:, jz * 128:(jz + 1) * 128]
                nc.tensor.matmul(z[:, 0:QB], lhsT, wp_sb[:, 0:QB],
                                 start=True, stop=True)
                nc.tensor.matmul(z[:, QB:2 * QB], lhsT, wp_sb[:, QB:2 * QB],
                                 start=True, stop=True)
                orows = outR[row0 + jz * 128:row0 + (jz + 1) * 128, :]
                o_sb = osp.tile([128, D], f32, name="o_sb", tag="o")
                if jz % 2 == 0:
                    nc.scalar.activation(o_sb[:], z[:], ACTF.Copy)
                else:
                    nc.vector.tensor_copy(o_sb[:], z[:])
                if use_bias:
                    nc.vector.tensor_tensor(
                        out=o_sb[:], in0=o_sb[:], in1=bp_bc[:], op=ADD,
                    )
                nc.sync.dma_start(out=orows, in_=o_sb[:])

            for b in range(B):
                for qb in range(N_QB):
                    qofs = b * T + qb * QB
                    p_y = [
                        ps_acc.tile([65, QB], f32, name=f"p_y{h}", tag="py")
                        for h in range(HPC)
                    ]
                    n_kt = 4 * (qb + 1)
                    zbase = max(n_kt - 4, 2)
                    pv_args = [None] * n_kt

                    def emit_pv(kt):
                        g_, q0_, e_mm_, st_, sp_ = pv_args[kt]
                        for h in range(HPC):
                            vbase = g_ * 130 + h * 65
                            nc.tensor.matmul(
                                p_y[h][:, q0_:QB], v_sb[:, vbase:vbase + 65],
                                e_mm_[:, h, :],
                                start=st_, stop=sp_,
                                skip_group_check=True,
                            )

                    for kt in range(n_kt):
                        g = b * N_KT_B + kt
                        ksl = slice(g * KTILE, (g + 1) * KTILE)
                        diag = kt - 4 * qb  # >= 0 on diagonal tiles
                        q0 = diag * 128 if diag >= 0 else 0
                        st = kt == 0
                        sp = kt == n_kt - 1
                        # both heads' scores -> one [128,1024] pair tile,
                        # restricted to the live q-range on diagonal tiles
                        p_s = ps_pair.tile([128, 2 * QB], f32, name="p_s", tag="pair")
                        nc.tensor.matmul(
                            p_s[:, q0:QB], kt_sb[0:DK, ksl],
                            qt_sb[0:DK, qofs + q0:qofs + QB],
                            start=True, stop=True,
                        )
                        nc.tensor.matmul(
                            p_s[:, QB + q0:2 * QB], kt_sb[DK:CPC, ksl],
                            qt_sb[DK:CPC, qofs + q0:qofs + QB],
                            start=True, stop=True,
                        )
                        # exp over both heads' live range. Engine routing:
                        # the first two kts of every block go to the Vector
                        # engine (Schraudolph bf16-bits exp as int16) so they
                        # are not queued behind the previous block's
                        # eviction+reciprocal chain on Scalar; later
                        # off-diagonal kts alternate Scalar(exact)/Vector,
                        # and later diagonal kts use exact Scalar exp.
                        # Renormalization cancels the approximation error.
                        use_vec = (kt == 0 or diag >= 2
                                   or (diag < 0 and kt % 5 in (1, 3)))
                        if not use_vec:
                            e_t = att.tile([128, 2 * QB], bf16, name="e_t", tag="et")
                            ps3 = p_s[:].rearrange("p (h q) -> p h q", q=QB)[:, :, q0:QB]
                            et3 = e_t[:].rearrange("p (h q) -> p h q", q=QB)[:, :, q0:QB]
                            nc.scalar.activation(et3, ps3, ACTF.Exp, scale=float(SCALE))
                            e_bf = e_t[:]
                        else:
                            e_i = att.tile([128, 2 * QB], mybir.dt.int16, name="e_i", tag="et")
                            if q0 == 0:
                                nc.vector.tensor_scalar(
                                    e_i[:], p_s[:],
                                    float(A16 * SCALE), float(C16),
                                    MUL, ADD,
                                )
                            else:
                                ei3 = e_i[:].rearrange("p (h q) -> p h q", q=QB)[:, :, q0:QB]
                                ps3 = p_s[:].rearrange("p (h q) -> p h q", q=QB)[:, :, q0:QB]
                                nc.vector.tensor_scalar(
                                    ei3, ps3,
                                    float(A16 * SCALE), float(C16),
                                    MUL, ADD,
                                )
                            e_bf = e_i[:].bitcast(bf16)
                        if diag >= 0:
                            # mask the triangular 128-col chunk; for the
                            # first two kts use Vector (same engine as the
                            # Schraudolph exp - no cross-engine hop on the
                            # P0/P1 critical path), else GpSimd
                            etm = e_bf.rearrange("p (h q) -> p h q", q=QB)[
                                :, :, q0:q0 + 128
                            ]
                            meng = nc.vector if kt < 2 else nc.gpsimd
                            meng.tensor_tensor(
                                out=etm,
                                in0=etm,
                                in1=tri[:][:, None, :].broadcast_to([128, HPC, 128]),
                                op=MUL,
                            )
                        e_mm = e_bf.rearrange("p (h q) -> p h q", q=QB)[:, :, q0:QB]
                        pv_args[kt] = (g, q0, e_mm, st, sp)
                        if kt == 1 and recip_pend:
                            srow_, rrow_, rrow_r_ = recip_pend.pop(0)
                            nc.vector.reciprocal_approx_fast(rrow_[:], srow_[:])
                            nc.scalar.activation(rrow_r_[:], rrow_[:], ACTF.Copy)
                        if kt >= 3:
                            emit_pv(kt - 3)
                        jz = kt - zbase
                        if 0 <= jz <= 3 and z_queue:
                            emit_z(z_queue[0][0], z_queue[0][1], jz)
                            if jz == 3:
                                z_queue.pop(0)
                    emit_pv(n_kt - 3)
                    # norm of the previous block here: its reciprocal row is
                    # a full block old, and the two outer-product matmuls
                    # fill the PE while the last exps complete.
                    if norm_queue:
                        emit_norm(*norm_queue.pop(0))
                    njz = n_kt - zbase  # z's already emitted in the k-loop
                    if z_queue and njz < 4:
                        emit_z(z_queue[0][0], z_queue[0][1], njz)
                        njz += 1
                    emit_pv(n_kt - 2)
                    if z_queue and njz < 4:
                        emit_z(z_queue[0][0], z_queue[0][1], njz)
                        njz += 1
                        if njz == 4:
                            z_queue.pop(0)
                    emit_pv(n_kt - 1)

                    # ---- block end: evict y + sums rows, then a 1-pass
                    # Scalar ACT reciprocal on the sums row. Scalar order
                    # [ysb0, srow0, recip] / vector [srow1, ysb1] frees the
                    # p_y banks fast AND finishes the reciprocal before the
                    # deferred outer-product needs it.
                    ysb = ysp.tile([128, QB], f32r, name="ysb", tag="ys")
                    srow = sbp.tile([1, 2 * QB], f32, name="srow", tag="sr")
                    nc.vector.tensor_copy(ysb[64:128, :], p_y[1][0:64, :])
                    nc.scalar.activation(ysb[0:64, :], p_y[0][0:64, :], ACTF.Copy)
                    nc.scalar.activation(srow[0:1, 0:QB], p_y[0][64:65, :], ACTF.Copy)
                    nc.scalar.activation(srow[0:1, QB:2 * QB], p_y[1][64:65, :], ACTF.Copy)
                    rrow = sbp.tile([1, 2 * QB], f32, name="rrow", tag="rr0")
                    rrow_r = sbp.tile([1, 2 * QB], f32r, name="rrow_r", tag="rrr")
                    recip_pend.append((srow, rrow, rrow_r))
                    norm_queue.append((ysb, rrow_r, qofs))

            # tail: flush ready z's first, then the last norm + its z's
            while recip_pend:
                srow_, rrow_, rrow_r_ = recip_pend.pop(0)
                nc.vector.reciprocal_approx_fast(rrow_[:], srow_[:])
                nc.vector.tensor_copy(rrow_r_[:], rrow_[:])
            while z_queue:
                y_, r_ = z_queue.pop(0)
                for jz in range(4):
                    emit_z(y_, r_, jz)
            while norm_queue:
                emit_norm(*norm_queue.pop(0))
                y_, r_ = z_queue.pop(0)
                for jz in range(4):
                    emit_z(y_, r_, jz)

            for _pool in (ps_acc, ps_pair, osp, rsp, sbp, ysp, att, vts):
                _pool.release()
            _free_v(); _free_kt(); _free_qt()
            xa.release()
            cst.release()

    nc.compile()
    return nc


_CACHED = {}


def _get_program(use_bias=False):
    if use_bias not in _CACHED:
        _CACHED[use_bias] = build_program(use_bias)
    return _CACHED[use_bias]


def make_in_maps(x, W_qkv, b_qkv, W_proj, b_proj):
    x = np.asarray(x, dtype=np.float32)
    W_qkv = np.asarray(W_qkv, dtype=np.float32)
    b_qkv = np.asarray(b_qkv, dtype=np.float32)
    W_proj = np.asarray(W_proj, dtype=np.float32)
    b_proj = np.asarray(b_proj, dtype=np.float32)

    import ml_dtypes

    bf = ml_dtypes.bfloat16
    # [p, rt, t, r]: per-partition rows are 8KB contiguous per rt-tile DMA
    xT = np.ascontiguousarray(
        x.reshape(ROWS, D).T.reshape(D // 128, 128, N_RT, RT)
        .transpose(1, 2, 0, 3)
    ).astype(bf)

    def wswz(w):  # [D, CPC] -> [p, t*m] so the weight DMA is contiguous
        return np.ascontiguousarray(
            w.reshape(D // 128, 128, CPC).transpose(1, 0, 2).reshape(128, D)
        )

    in_maps = []
    for c in range(N_CORES):
        ch = c * CPC  # channel offset of this core's heads
        wq_c = wswz(W_qkv[:, ch:ch + CPC]).astype(bf)
        wk_c = wswz(W_qkv[:, D + ch:D + ch + CPC]).astype(bf)
        wv_c = wswz(W_qkv[:, 2 * D + ch:2 * D + ch + CPC]).astype(bf)
        wp_c = round_f32r(W_proj[ch:ch + CPC, :])
        bqkv_c = np.stack(
            [b_qkv[ch:ch + CPC], b_qkv[D + ch:D + ch + CPC], b_qkv[2 * D + ch:2 * D + ch + CPC]],
            axis=1,
        ).astype(np.float32)
        # b_proj added once (core 0 only); partials are summed on host
        bp_c = (
            b_proj.reshape(1, D)
            if c == 0
            else np.zeros((1, D), np.float32)
        )
        in_maps.append(
            {
                "xT": xT,
                "wq": np.ascontiguousarray(wq_c),
                "wk": np.ascontiguousarray(wk_c),
                "wv": np.ascontiguousarray(wv_c),
                "wp": np.ascontiguousarray(wp_c),
                "bqkv": np.ascontiguousarray(bqkv_c),
                "bp": np.ascontiguousarray(bp_c.astype(np.float32)),
            }
        )
    return in_maps


def run(nc, in_maps, trace=False, trace_kwargs=None):
    from concourse.bass_utils import run_bass_kernel_spmd

    return run_bass_kernel_spmd(
        nc,
        in_maps,
        core_ids=list(range(N_CORES)),
        trace=trace,
        **(trace_kwargs or {}),
    )


def gather_output(results):
    acc = results[0]["outR"].astype(np.float32)
    for r in results[1:]:
        acc = acc + r["outR"]
    return acc.reshape(B, T, D)


def kernel(x, W_qkv, b_qkv, W_proj, b_proj):
    use_bias = bool(np.any(np.asarray(b_qkv)) or np.any(np.asarray(b_proj)))
    nc = _get_program(use_bias)
    in_maps = make_in_maps(x, W_qkv, b_qkv, W_proj, b_proj)
    res = run(nc, in_maps, trace=False)
    return gather_output(res.results)


# revision 38
# speedup vs baseline: 1.0073x; 1.0051x over previous
"""Causal self-attention (dense transformer block) on 8 Trainium2 NeuronCores.

Sharding (Megatron-style tensor parallel over heads):
  - 16 heads, 8 cores -> 2 heads/core. Each core computes the qkv projection
    for its 2 heads (column-sharded W_qkv), causal attention for those heads
    over all 4 batches, and a row-sharded c_proj partial. The host sums the
    8 partial outputs (the row-parallel unshard).
  - Softmax: scores are O(+-6) so exp() without max-subtraction is exact in
    fp32; row sums come free from the PV matmul via a ones-column appended
    to V ([V|1]); causal masking is a 0/1 multiply restricted to the single
    triangular 128-col chunk of each diagonal k-tile.
  - The PE is the bottleneck (and downclocks after idle gaps: 0.65/1.2/2.4
    GHz p-states), so phase 2 is scheduled to keep it continuously busy:
    * k-loop is software-pipelined: scores(kt) issue before PV(kt-1), so
      the exp of kt runs while the PE computes other matmuls.
    * exps split across Scalar ACT (exact, diagonal + some off-diagonal
      tiles) and Vector DVE (Schraudolph bf16-bit exp as int16) so the two
      engines exp concurrently and neither gates the PE.
    * diagonal k-tiles only compute the live q-range [128j, 512): scores,
      exp and PV all shrink; the mask multiply is one [128,2,128] op.
    * y is normalized BEFORE c_proj (sums row broadcast via GpSimd
      partition_broadcast, full-lane DVE reciprocal, two tensor_tensor
      multiplies), so c_proj is a single K=128 f32r matmul per
      (q-chunk, oc-half) -- half the matmuls of the split-head form and no
      PSUM merge arithmetic on the Scalar/Vector engines.
    * c_proj for block i is interleaved into the tail of block i+1's
      k-loop (one z-pair per kt) so PSUM pair slots rotate without stalls.
  - x and the qkv weights are bf16; q/k/v are bf16 downstream. c_proj runs
    in f32r (full PE rate at N=512).
  - Phase 1 (qkv projection) pipelines the V PE-transposes one row-tile
    behind the matmuls so the PE never waits on PSUM evictions.
"""

import sys

sys.path.insert(0, "/opt/trn_rl_repo")

import numpy as np

N_CORES = 8
B, T, D = 4, 2048, 1024
H, DK = 16, 64
HPC = H // N_CORES            # heads per core = 2
CPC = HPC * DK                # channels per core = 128
ROWS = B * T                  # 8192
RT = 512                      # row-tile (free dim) for projections
N_RT = ROWS // RT             # 16
KTILE = 128                   # key tile
QB = 512                      # query block
N_QB = T // QB                # 4 query blocks per batch
N_KT_B = T // KTILE           # 16 key tiles per batch
SCALE = 1.0 / np.sqrt(DK)
# Schraudolph exp for bf16 bit patterns: bf16_bits(exp(x)) ~ A16*x + C16
A16 = 128.0 / np.log(2.0)
C16 = 16252.0  # 127*2^7 with bias correction (halves the sawtooth error)


def round_f32r(x):
    """Round fp32 -> fp32r (round-to-nearest-even at 11 fraction bits)."""
    b = np.ascontiguousarray(x, dtype=np.float32).view(np.uint32)
    r = ((b.astype(np.uint64) + 0x7FF + ((b >> 12) & 1)) & ~np.uint64(0xFFF)).astype(
        np.uint32
    )
    return r.view(np.float32)


def build_program(use_bias=False):
    import concourse.bass as bass  # noqa: F401
    import concourse.mybir as mybir
    import concourse.tile as tile
    from concourse import bacc
    from concourse.masks import make_identity

    f32 = mybir.dt.float32
    f32r = mybir.dt.float32r
    bf16 = mybir.dt.bfloat16
    ACTF = mybir.ActivationFunctionType
    MUL = mybir.AluOpType.mult
    ADD = mybir.AluOpType.add

    nc = bacc.Bacc(None, target_bir_lowering=False)
    with tile.TileContext(nc) as tc:
        with tc.tile_pool(name="dram", bufs=1, space="DRAM") as dram:
            # xT pre-swizzled on host to [p, rt, t, r]; weights to [p, t*m]
            # so every DMA is long contiguous runs per partition
            xT = dram.tile([128, N_RT, D // 128, RT], bf16, kind="ExternalInput", name="xT", uniquify=False)
            wq = dram.tile([128, D], bf16, kind="ExternalInput", name="wq", uniquify=False)
            wk = dram.tile([128, D], bf16, kind="ExternalInput", name="wk", uniquify=False)
            wv = dram.tile([128, D], bf16, kind="ExternalInput", name="wv", uniquify=False)
            wp = dram.tile([CPC, D], f32r, kind="ExternalInput", name="wp", uniquify=False)
            bqkv = dram.tile([CPC, 3], f32, kind="ExternalInput", name="bqkv", uniquify=False)
            bp = dram.tile([1, D], f32, kind="ExternalInput", name="bp", uniquify=False)
            outR = dram.tile([ROWS, D], f32, kind="ExternalOutput", name="outR", uniquify=False)

            # ---------------- constants / weights in SBUF ----------------
            cst = tc.alloc_tile_pool(name="cst", bufs=1)

            # x tiles pool first: issue the rt=0/1 input DMAs before the
            # weight DMAs so the first qkv matmul starts ASAP.
            nkt = D // 128
            xa = tc.alloc_tile_pool(name="xa", bufs=3)
            xt_pre = []
            for rt in range(2):
                rsl = slice(rt * RT, (rt + 1) * RT)
                xt = xa.tile([128, nkt, RT], bf16, name="xt", tag="xt")
                xsrc = xT[:, rt, :, :]
                if rt == 0:
                    # 4 piecewise DMAs: the first 2 k-chunks land fast so the
                    # first matmuls start without waiting for the full tile
                    for c in range(4):
                        nc.sync.dma_start(
                            out=xt[:, 2 * c:2 * c + 2, :],
                            in_=xsrc[:, 2 * c:2 * c + 2, :],
                        )
                else:
                    nc.sync.dma_start(out=xt[:], in_=xsrc)
                xt_pre.append(xt)

            wq_sb = cst.tile([128, D], bf16, name="wq_sb")
            wk_sb = cst.tile([128, D], bf16, name="wk_sb")
            wv_sb = cst.tile([128, D], bf16, name="wv_sb")
            for w_dram, w_sb in ((wq, wq_sb), (wk, wk_sb), (wv, wv_sb)):
                nc.sync.dma_start(out=w_sb[:], in_=w_dram[:])
            wp_sb = cst.tile([CPC, D], f32r, name="wp_sb")
            nc.sync.dma_start(out=wp_sb[:], in_=wp[:])
            bqkv_sb = cst.tile([CPC, 3], f32, name="bqkv_sb")
            nc.sync.dma_start(out=bqkv_sb[:], in_=bqkv[:])
            bp_sb = cst.tile([1, D], f32, name="bp_sb")
            nc.sync.dma_start(out=bp_sb[:], in_=bp[:])
            bp_bc = cst.tile([128, D], f32, name="bp_bc")
            if use_bias:
                nc.gpsimd.partition_broadcast(bp_bc[:], bp_sb[:])

            ident32 = cst.tile([128, 128], f32, name="ident32")
            make_identity(nc, ident32)
            ident = cst.tile([128, 128], f32r, name="ident")
            nc.vector.tensor_copy(ident[:], ident32[:])

            # one [128,128] triangular mask: keep where q >= k (within-chunk)
            tri = cst.tile([128, 128], bf16, name="tri")
            mscratch = cst.tile([128, 128], f32, name="mscratch")
            nc.gpsimd.memset(mscratch[:], 1.0)
            nc.gpsimd.affine_select(
                out=mscratch[:],
                in_=mscratch[:],
                compare_op=mybir.AluOpType.is_ge,
                fill=0.0,
                base=0,
                pattern=[[1, 128]],
                channel_multiplier=-1,
            )
            nc.vector.tensor_copy(tri[:], mscratch[:])

            # ---------------- long-lived activations ----------------
            qt_sb, _free_qt = tc.tile([CPC, ROWS], bf16, name="qt_sb")
            kt_sb, _free_kt = tc.tile([CPC, ROWS], bf16, name="kt_sb")
            # V tiles: per key-tile g: [128 keys, 130]: h0 V|1 at cols 0:65,
            # h1 V|1 at cols 65:130 (ones columns pre-set once)
            v_sb, _free_v = tc.tile([128, (ROWS // KTILE) * 130], bf16, name="v_sb")
            nc.gpsimd.memset(v_sb[:], 1.0)

            # ---------------- pools ----------------
            vts = tc.alloc_tile_pool(name="vts", bufs=4)
            att = tc.alloc_tile_pool(name="att", bufs=8)     # e_t [128,1024]
            ysp = tc.alloc_tile_pool(name="ysp", bufs=3)     # ysb [128,512]
            sbp = tc.alloc_tile_pool(name="sbp", bufs=3)     # srow/rrow rows
            rsp = tc.alloc_tile_pool(name="rsp", bufs=2)     # rr_sb [128,1024]
            osp = tc.alloc_tile_pool(name="osp", bufs=6)     # o_sb [128,1024]
            # phase 1 gets its own 4-slot pair pool (8 banks): q/k/v psum
            # evictions never gate the next row-tile's matmuls
            ps1 = tc.alloc_tile_pool(name="ps1", bufs=4, space="PSUM")

            # ================= phase 1: qkv projections =================
            # V transposes for rt are deferred until after rt+1's matmuls so
            # the PE never waits on the vt eviction chain.
            def emit_transpose(vt_t, rt):
                p_tr = ps1.tile([128, RT], f32r, name="p_tr", tag="pair")
                for c4 in range(RT // 128):
                    nc.tensor.transpose(
                        p_tr[:, c4 * 128:(c4 + 1) * 128],
                        vt_t[:, c4 * 128:(c4 + 1) * 128],
                        ident[:],
                    )
                for c4 in range(RT // 128):
                    g = rt * (RT // 128) + c4
                    base = g * 130
                    nc.vector.tensor_copy(
                        v_sb[:, base:base + 130]
                        .rearrange("p (h c) -> p h c", c=65)[:, :, 0:64],
                        p_tr[:, c4 * 128:(c4 + 1) * 128]
                        .rearrange("p (h c) -> p h c", c=64),
                    )

            pend_tr = None
            for rt in range(N_RT):
                rsl = slice(rt * RT, (rt + 1) * RT)
                if rt < 2:
                    xt = xt_pre[rt]
                else:
                    xt = xa.tile([128, nkt, RT], bf16, name="xt", tag="xt")
                    nc.sync.dma_start(out=xt[:], in_=xT[:, rt, :, :])
                xts = [xt[:, kt, :] for kt in range(nkt)]
                # q and k share one [128,1024] psum pair (separate bank halves)
                p_qk = ps1.tile([CPC, 2 * RT], f32, name="p_qk", tag="pair")
                p_v = ps1.tile([CPC, RT], f32, name="p_v", tag="pair")
                for kt in range(nkt):
                    ksl = slice(kt * 128, (kt + 1) * 128)
                    st = kt == 0
                    sp = kt == nkt - 1
                    nc.tensor.matmul(p_qk[:, 0:RT], wq_sb[:, ksl], xts[kt], start=st, stop=sp)
                    nc.tensor.matmul(p_qk[:, RT:2 * RT], wk_sb[:, ksl], xts[kt], start=st, stop=sp)
                    nc.tensor.matmul(p_v[:], wv_sb[:, ksl], xts[kt], start=st, stop=sp)
                # evict Q^T, K^T
                if use_bias:
                    nc.vector.tensor_scalar_add(qt_sb[:, rsl], p_qk[:, 0:RT], bqkv_sb[:, 0:1])
                    nc.vector.tensor_scalar_add(kt_sb[:, rsl], p_qk[:, RT:2 * RT], bqkv_sb[:, 1:2])
                else:
                    nc.scalar.activation(qt_sb[:, rsl], p_qk[:, 0:RT], ACTF.Copy)
                    nc.vector.tensor_copy(kt_sb[:, rsl], p_qk[:, RT:2 * RT])
                # V^T -> SBUF (with bias); PE-transpose deferred one rt
                vt_t = vts.tile([CPC, RT], f32r, name="vt_t", tag="vt")
                if use_bias:
                    nc.vector.tensor_scalar_add(vt_t[:], p_v[:], bqkv_sb[:, 2:3])
                else:
                    nc.scalar.activation(vt_t[:], p_v[:], ACTF.Copy)
                if pend_tr is not None:
                    emit_transpose(*pend_tr)
                pend_tr = (vt_t, rt)
            emit_transpose(*pend_tr)
            ps1.release()
            # PSUM: pair pool 3 x 2 banks (6) + p_y 2 x 1 bank (2) = 8 banks
            ps_pair = tc.alloc_tile_pool(name="ps_pair", bufs=3, space="PSUM")
            ps_acc = tc.alloc_tile_pool(name="ps_acc", bufs=2, space="PSUM")

            # ================= phase 2: causal attention =================
            # Two-block software pipeline keeps every c_proj dependency at
            # least one full block old by the time the PE reaches it:
            #   norm_queue: (ysb_raw, rrow, row0) -> at kt=0 of the next
            #     block, a PE outer-product broadcasts 1/s into PSUM and two
            #     DVE multiplies normalize ysb in place.
            #   z_queue: (ysb_norm, row0) -> c_proj z-pairs interleaved into
            #     the k-loop one further block later.
            norm_queue = []
            z_queue = []
            recip_pend = []
            ones_f = cst.tile([1, 128], f32, name="ones_f")
            nc.gpsimd.memset(ones_f[:], 1.0)
            ones128 = cst.tile([1, 128], f32r, name="ones128")
            nc.vector.tensor_copy(ones128[:], ones_f[:])

            def emit_norm(ysb, rrow, row0):
                rr_ps = ps_pair.tile([128, 2 * QB], f32, name="rr_ps", tag="pair")
                nc.tensor.matmul(rr_ps[:, 0:QB], ones128[:], rrow[0:1, 0:QB],
                                 start=True, stop=True)
                nc.tensor.matmul(rr_ps[:, QB:2 * QB], ones128[:], rrow[0:1, QB:2 * QB],
                                 start=True, stop=True)
                # evict the broadcast to SBUF fast (frees the PSUM slot for
                # the next block's scores) and multiply on GpSimd -- keeps
                # the Vector queue clear for the next block's first exps
                rr_sb = rsp.tile([128, 2 * QB], f32, name="rr_sb", tag="rs")
                nc.scalar.activation(rr_sb[:, 0:QB], rr_ps[:, 0:QB], ACTF.Copy)
                nc.vector.tensor_copy(rr_sb[:, QB:2 * QB], rr_ps[:, QB:2 * QB])
                nc.gpsimd.tensor_tensor(
                    out=ysb[0:64, :], in0=ysb[0:64, :],
                    in1=rr_sb[0:64, 0:QB], op=MUL,
                )
                nc.gpsimd.tensor_tensor(
                    out=ysb[64:128, :], in0=ysb[64:128, :],
                    in1=rr_sb[64:128, QB:2 * QB], op=MUL,
                )
                z_queue.append((ysb, row0))

            def emit_z(y_n, row0, jz):
                z = ps_pair.tile([128, 2 * QB], f32, name="z", tag="pair")
                lhsT = y_n[:, jz * 128:(jz + 1) * 128]
                nc.tensor.matmul(z[:, 0:QB], lhsT, wp_sb[:, 0:QB],
                                 start=True, stop=True)
                nc.tensor.matmul(z[:, QB:2 * QB], lhsT, wp_sb[:, QB:2 * QB],
                                 start=True, stop=True)
                orows = outR[row0 + jz * 128:row0 + (jz + 1) * 128, :]
                o_sb = osp.tile([128, D], f32, name="o_sb", tag="o")
                if jz % 2 == 0:
                    nc.scalar.activation(o_sb[:], z[:], ACTF.Copy)
                else:
                    nc.vector.tensor_copy(o_sb[:], z[:])
                if use_bias:
                    nc.vector.tensor_tensor(
                        out=o_sb[:], in0=o_sb[:], in1=bp_bc[:], op=ADD,
                    )
                nc.sync.dma_start(out=orows, in_=o_sb[:])

            for b in range(B):
                for qb in range(N_QB):
                    qofs = b * T + qb * QB
                    p_y = [
                        ps_acc.tile([65, QB], f32, name=f"p_y{h}", tag="py")
                        for h in range(HPC)
                    ]
                    n_kt = 4 * (qb + 1)
                    zbase = max(n_kt - 4, 2)
                    pv_args = [None] * n_kt

                    def emit_pv(kt):
                        g_, q0_, e_mm_, st_, sp_ = pv_args[kt]
                        for h in range(HPC):
                            vbase = g_ * 130 + h * 65
                            nc.tensor.matmul(
                                p_y[h][:, q0_:QB], v_sb[:, vbase:vbase + 65],
                                e_mm_[:, h, :],
                                start=st_, stop=sp_,
                                skip_group_check=True,
                            )

                    for kt in range(n_kt):
                        g = b * N_KT_B + kt
                        ksl = slice(g * KTILE, (g + 1) * KTILE)
                        diag = kt - 4 * qb  # >= 0 on diagonal tiles
                        q0 = diag * 128 if diag >= 0 else 0
                        st = kt == 0
                        sp = kt == n_kt - 1
                        # both heads' scores -> one [128,1024] pair tile,
                        # restricted to the live q-range on diagonal tiles
                        p_s = ps_pair.tile([128, 2 * QB], f32, name="p_s", tag="pair")
                        nc.tensor.matmul(
                            p_s[:, q0:QB], kt_sb[0:DK, ksl],
                            qt_sb[0:DK, qofs + q0:qofs + QB],
                            start=True, stop=True,
                        )
                        nc.tensor.matmul(
                            p_s[:, QB + q0:2 * QB], kt_sb[DK:CPC, ksl],
                            qt_sb[DK:CPC, qofs + q0:qofs + QB],
                            start=True, stop=True,
                        )
                        # exp over both heads' live range. Engine routing:
                        # the first two kts of every block go to the Vector
                        # engine (Schraudolph bf16-bits exp as int16) so they
                        # are not queued behind the previous block's
                        # eviction+reciprocal chain on Scalar; later
                        # off-diagonal kts alternate Scalar(exact)/Vector,
                        # and later diagonal kts use exact Scalar exp.
                        # Renormalization cancels the approximation error.
                        use_vec = (kt == 0 or diag >= 2
                                   or (diag < 0 and kt % 5 in (1, 3)))
                        if not use_vec:
                            e_t = att.tile([128, 2 * QB], bf16, name="e_t", tag="et")
                            ps3 = p_s[:].rearrange("p (h q) -> p h q", q=QB)[:, :, q0:QB]
                            et3 = e_t[:].rearrange("p (h q) -> p h q", q=QB)[:, :, q0:QB]
                            nc.scalar.activation(et3, ps3, ACTF.Exp, scale=float(SCALE))
                            e_bf = e_t[:]
                        else:
                            e_i = att.tile([128, 2 * QB], mybir.dt.int16, name="e_i", tag="et")
                            if q0 == 0:
                                nc.vector.tensor_scalar(
                                    e_i[:], p_s[:],
                                    float(A16 * SCALE), float(C16),
                                    MUL, ADD,
                                )
                            else:
                                ei3 = e_i[:].rearrange("p (h q) -> p h q", q=QB)[:, :, q0:QB]
                                ps3 = p_s[:].rearrange("p (h q) -> p h q", q=QB)[:, :, q0:QB]
                                nc.vector.tensor_scalar(
                                    ei3, ps3,
                                    float(A16 * SCALE), float(C16),
                                    MUL, ADD,
                                )
                            e_bf = e_i[:].bitcast(bf16)
                        if diag >= 0:
                            # mask the triangular 128-col chunk; for the
                            # first two kts use Vector (same engine as the
                            # Schraudolph exp - no cross-engine hop on the
                            # P0/P1 critical path), else GpSimd
                            etm = e_bf.rearrange("p (h q) -> p h q", q=QB)[
                                :, :, q0:q0 + 128
                            ]
                            meng = nc.vector if kt < 2 else nc.gpsimd
                            meng.tensor_tensor(
                                out=etm,
                                in0=etm,
                                in1=tri[:][:, None, :].broadcast_to([128, HPC, 128]),
                                op=MUL,
                            )
                        e_mm = e_bf.rearrange("p (h q) -> p h q", q=QB)[:, :, q0:QB]
                        pv_args[kt] = (g, q0, e_mm, st, sp)
                        if kt == 1 and recip_pend:
                            srow_, rrow_, rrow_r_ = recip_pend.pop(0)
                            nc.vector.reciprocal_approx_fast(rrow_[:], srow_[:])
                            nc.scalar.activation(rrow_r_[:], rrow_[:], ACTF.Copy)
                        if kt >= 3:
                            emit_pv(kt - 3)
                        jz = kt - zbase
                        if 0 <= jz <= 3 and z_queue:
                            emit_z(z_queue[0][0], z_queue[0][1], jz)
                            if jz == 3:
                                z_queue.pop(0)
                    emit_pv(n_kt - 3)
                    # norm of the previous block here: its reciprocal row is
                    # a full block old, and the two outer-product matmuls
                    # fill the PE while the last exps complete.
                    if norm_queue:
                        emit_norm(*norm_queue.pop(0))
                    njz = n_kt - zbase  # z's already emitted in the k-loop
                    if z_queue and njz < 4:
                        emit_z(z_queue[0][0], z_queue[0][1], njz)
                        njz += 1
                    emit_pv(n_kt - 2)
                    if z_queue and njz < 4:
                        emit_z(z_queue[0][0], z_queue[0][1], njz)
                        njz += 1
                        if njz == 4:
                            z_queue.pop(0)
                    emit_pv(n_kt - 1)

                    # ---- block end: evict y + sums rows, then a 1-pass
                    # Scalar ACT reciprocal on the sums row. Scalar order
                    # [ysb0, srow0, recip] / vector [srow1, ysb1] frees the
                    # p_y banks fast AND finishes the reciprocal before the
                    # deferred outer-product needs it.
                    ysb = ysp.tile([128, QB], f32r, name="ysb", tag="ys")
                    srow = sbp.tile([1, 2 * QB], f32, name="srow", tag="sr")
                    nc.vector.tensor_copy(ysb[64:128, :], p_y[1][0:64, :])
                    nc.scalar.activation(ysb[0:64, :], p_y[0][0:64, :], ACTF.Copy)
                    nc.scalar.activation(srow[0:1, 0:QB], p_y[0][64:65, :], ACTF.Copy)
                    nc.scalar.activation(srow[0:1, QB:2 * QB], p_y[1][64:65, :], ACTF.Copy)
                    rrow = sbp.tile([1, 2 * QB], f32, name="rrow", tag="rr0")
                    rrow_r = sbp.tile([1, 2 * QB], f32r, name="rrow_r", tag="rrr")
                    recip_pend.append((srow, rrow, rrow_r))
                    norm_queue.append((ysb, rrow_r, qofs))

            # tail: flush ready z's first, then the last norm + its z's
            while recip_pend:
                srow_, rrow_, rrow_r_ = recip_pend.pop(0)
                nc.vector.reciprocal_approx_fast(rrow_[:], srow_[:])
                nc.vector.tensor_copy(rrow_r_[:], rrow_[:])
            while z_queue:
                y_, r_ = z_queue.pop(0)
                for jz in range(4):
                    emit_z(y_, r_, jz)
            while norm_queue:
                emit_norm(*norm_queue.pop(0))
                y_, r_ = z_queue.pop(0)
                for jz in range(4):
                    emit_z(y_, r_, jz)

            for _pool in (ps_acc, ps_pair, osp, rsp, sbp, ysp, att, vts):
                _pool.release()
            _free_v(); _free_kt(); _free_qt()
            xa.release()
            cst.release()

    nc.compile()
    return nc


_CACHED = {}


def _get_program(use_bias=False):
    if use_bias not in _CACHED:
        _CACHED[use_bias] = build_program(use_bias)
    return _CACHED[use_bias]


def make_in_maps(x, W_qkv, b_qkv, W_proj, b_proj):
    x = np.asarray(x, dtype=np.float32)
    W_qkv = np.asarray(W_qkv, dtype=np.float32)
    b_qkv = np.asarray(b_qkv, dtype=np.float32)
    W_proj = np.asarray(W_proj, dtype=np.float32)
    b_proj = np.asarray(b_proj, dtype=np.float32)

    import ml_dtypes

    bf = ml_dtypes.bfloat16
    # [p, rt, t, r]: per-partition rows are 8KB contiguous per rt-tile DMA
    xT = np.ascontiguousarray(
        x.reshape(ROWS, D).T.reshape(D // 128, 128, N_RT, RT)
        .transpose(1, 2, 0, 3)
    ).astype(bf)

    def wswz(w):  # [D, CPC] -> [p, t*m] so the weight DMA is contiguous
        return np.ascontiguousarray(
            w.reshape(D // 128, 128, CPC).transpose(1, 0, 2).reshape(128, D)
        )

    in_maps = []
    for c in range(N_CORES):
        ch = c * CPC  # channel offset of this core's heads
        wq_c = wswz(W_qkv[:, ch:ch + CPC]).astype(bf)
        wk_c = wswz(W_qkv[:, D + ch:D + ch + CPC]).astype(bf)
        wv_c = wswz(W_qkv[:, 2 * D + ch:2 * D + ch + CPC]).astype(bf)
        wp_c = round_f32r(W_proj[ch:ch + CPC, :])
        bqkv_c = np.stack(
            [b_qkv[ch:ch + CPC], b_qkv[D + ch:D + ch + CPC], b_qkv[2 * D + ch:2 * D + ch + CPC]],
            axis=1,
        ).astype(np.float32)
        # b_proj added once (core 0 only); partials are summed on host
        bp_c = (
            b_proj.reshape(1, D)
            if c == 0
            else np.zeros((1, D), np.float32)
        )
        in_maps.append(
            {
                "xT": xT,
                "wq": np.ascontiguousarray(wq_c),
                "wk": np.ascontiguousarray(wk_c),
                "wv": np.ascontiguousarray(wv_c),
                "wp": np.ascontiguousarray(wp_c),
                "bqkv": np.ascontiguousarray(bqkv_c),
                "bp": np.ascontiguousarray(bp_c.astype(np.float32)),
            }
        )
    return in_maps


def run(nc, in_maps, trace=False, trace_kwargs=None):
    from concourse.bass_utils import run_bass_kernel_spmd

    return run_bass_kernel_spmd(
        nc,
        in_maps,
        core_ids=list(range(N_CORES)),
        trace=trace,
        **(trace_kwargs or {}),
    )


def gather_output(results):
    acc = results[0]["outR"].astype(np.float32)
    for r in results[1:]:
        acc = acc + r["outR"]
    return acc.reshape(B, T, D)


def kernel(x, W_qkv, b_qkv, W_proj, b_proj):
    use_bias = bool(np.any(np.asarray(b_qkv)) or np.any(np.asarray(b_proj)))
    nc = _get_program(use_bias)
    in_maps = make_in_maps(x, W_qkv, b_qkv, W_proj, b_proj)
    res = run(nc, in_maps, trace=False)
    return gather_output(res.results)
